# revision 1
# baseline (speedup 1.0000x reference)
"""Trainium2 Bass/Tile kernel for nn_FB_FMM (sparse_attention).

Computation (per batch element b, with N = H*W = 4096 tokens, C=256, D=32):
  1. Self-attention:  sa_out = attn(conv(x,sa_wq), conv(x,sa_wk), conv(x,sa_wv))
     x' = sa_gamma * sa_out + x
  2. Masked cross-attention (FB_FMM):
     ff = mask * x'; fb = (1-mask) * x'
     sw_bg = attn(conv(ff,wq), conv(fb,wk), conv(fb,wv))
     out = x' + gamma * ff * (std(sw_bg)/std(ff))    [per-channel std, ddof=1]

Sharding: 8 cores = 2 batch groups x 4-way query-row sharding (1024 rows each).
Each core computes its row-chunk of both attention layers; K/V sides are
computed redundantly per core (cheap: D=32 / one C x C conv). Cross-core
communication inside the kernel:
  - AllGather of x' chunks within each 4-core batch group (layer-2 K/V need
    the full x'), split into two 512-row phases so the first overlaps the
    second half of the layer-1 attention loop.
  - AllReduce of per-channel [sum, sumsq] stats for the FMM std ratio.

Layouts: feature maps are channel-major (C on partitions). Scores are computed
transposed (S^T: keys j on partitions, queries i free; logits are small so exp
needs no max-subtraction pass). The AV matmul keeps V^T slices stationary
(weight reuse) with E^T moving, producing O in natural (c x i) layout; the
softmax denominator comes from one extra M=1 ones-matmul per tile, and the
reciprocal row is broadcast across partitions with a K=1 ones matmul. All
heavy matmuls run in float32r (1 cycle/row vs 4 for fp32; ~1e-4 relative
rounding). V-conv biases are folded out mathematically (sum_j A[i,j] = 1
makes the layer-1 V bias a constant shift folded into the residual; variance
is shift-invariant so the layer-2 V bias drops out of the FMM std).
"""

import numpy as np

P = 128
B, C, HH, WW = 2, 256, 64, 64
N = HH * WW            # 4096 tokens
D = 32                 # q/k channels
NCORES = 8
RSH = 4                # row shards per batch group
R = N // RSH           # 1024 query rows per core
NT = N // P            # 32 key tiles
IC = 512               # query i-chunk (one PSUM bank of fp32)
EPS = 1e-5
F32 = np.float32

_CACHE = {}


def _build_bass():
    """Build the Bass/Tile program (single SPMD NEFF for all 8 cores)."""
    import concourse.bass as bass
    from concourse import bacc, mybir, tile

    f32 = mybir.dt.float32
    f32r = mybir.dt.float32r
    bf16 = mybir.dt.bfloat16
    AX = mybir.AxisListType
    OP = mybir.AluOpType
    AF = mybir.ActivationFunctionType

    nc = bacc.Bacc(
        "TRN2", target_bir_lowering=False, debug=False, num_devices=NCORES
    )
    bf16d = mybir.dt.bfloat16

    # ---------------- I/O ----------------
    xf_d = nc.dram_tensor("xf", [C, N], bf16d, kind="ExternalInput")
    xc_d = nc.dram_tensor("xc", [C, R], f32r, kind="ExternalInput")
    mrow_d = nc.dram_tensor("mrow", [1, N], f32, kind="ExternalInput")
    mcrow_d = nc.dram_tensor("mcrow", [1, R], f32, kind="ExternalInput")
    wqT1_d = nc.dram_tensor("wqT1", [C, D], f32r, kind="ExternalInput")
    wkT1_d = nc.dram_tensor("wkT1", [C, D], bf16d, kind="ExternalInput")
    wvT1_d = nc.dram_tensor("wvT1", [C, C], bf16d, kind="ExternalInput")
    wqT2_d = nc.dram_tensor("wqT2", [C, D], f32r, kind="ExternalInput")
    wkT2_d = nc.dram_tensor("wkT2", [C, D], bf16d, kind="ExternalInput")
    wvT2_d = nc.dram_tensor("wvT2", [C, C], bf16d, kind="ExternalInput")
    # consts columns: 0 sa_gamma, 1 gamma, 2/3 sa_gamma*sa_bv halves,
    # 6 sa_bq, 7 sa_bk, 8 bq, 9 bk (cols 6-9 live on partitions 0..31)
    consts_d = nc.dram_tensor("consts", [P, 10], f32, kind="ExternalInput")
    out_d = nc.dram_tensor("outc", [C, R], f32, kind="ExternalOutput")

    groups = [[0, 1, 2, 3], [4, 5, 6, 7]]

    with tile.TileContext(nc) as tc:
        from contextlib import ExitStack

        ctx = ExitStack()
        with ctx:
            big = ctx.enter_context(tc.tile_pool(name="big", bufs=1))
            epool = ctx.enter_context(tc.tile_pool(name="epool", bufs=4))
            onpool = ctx.enter_context(tc.tile_pool(name="onpool", bufs=3))
            sqpool = ctx.enter_context(tc.tile_pool(name="sqpool", bufs=2))
            fbpool = ctx.enter_context(tc.tile_pool(name="fbpool", bufs=4))
            rcpool = ctx.enter_context(tc.tile_pool(name="rcpool", bufs=4))
            finpool = ctx.enter_context(tc.tile_pool(name="finpool", bufs=2))
            misc = ctx.enter_context(tc.tile_pool(name="misc", bufs=1))
            psA = ctx.enter_context(
                tc.tile_pool(name="psA", bufs=2, space="PSUM")
            )
            psS = ctx.enter_context(
                tc.tile_pool(name="psS", bufs=3, space="PSUM")
            )
            psO = ctx.enter_context(
                tc.tile_pool(name="psO", bufs=3, space="PSUM")
            )
            dram = ctx.enter_context(
                tc.tile_pool(name="dram", bufs=1, space="DRAM")
            )

            # ------------- persistent SBUF tiles -------------
            xc_sb = big.tile([P, 2, R], f32r, tag="xc", name="xc_sb")
            mask_sb = big.tile([P, N], f32, tag="mask", name="mask_sb")
            maskc_sb = big.tile([P, R], f32, tag="maskc", name="maskc_sb")
            xp_sb = big.tile([P, 2, R], f32, tag="xp", name="xp_sb")
            xp16_sb = big.tile([P, 2, R], bf16, tag="xp16", name="xp16_sb")
            ff_sb = big.tile([P, 2, R], f32r, tag="ff", name="ff_sb")
            wqT1_sb = big.tile([P, 2, D], f32r, tag="wqT1", name="wqT1_sb")
            wkT1_sb = big.tile([P, 2, D], bf16, tag="wkT1", name="wkT1_sb")
            wvT1_sb = big.tile([P, 2, C], bf16, tag="wvT1", name="wvT1_sb")
            wqT2_sb = big.tile([P, 2, D], f32r, tag="wqT2", name="wqT2_sb")
            wkT2_sb = big.tile([P, 2, D], bf16, tag="wkT2", name="wkT2_sb")
            wvT2_sb = big.tile([P, 2, C], bf16, tag="wvT2", name="wvT2_sb")
            consts_sb = big.tile([P, 10], f32, tag="consts", name="consts_sb")
            # ones column (f32r) for the denominator matmul; ones row (f32)
            # for the K=1 reciprocal-replication matmul
            onesc_sb = big.tile([P, 1], bf16, tag="onesc", name="onesc_sb")
            onesr_sb = big.tile([1, P], f32r, tag="onesr", name="onesr_sb")
            stats_sb = misc.tile([P, 8], f32, tag="stats", name="stats_sb")

            # input DMAs: small tensors first on the HWDGE queue; x-full
            # and the bf16 weight casts go through gpsimd (casting DMAs run
            # on a separate queue and overlap)
            xf_sb = big.tile([P, 2, N], bf16, tag="xbig", name="xf_sb")
            for k in range(2):
                for jc in range(4):
                    js = slice(jc * (N // 4), (jc + 1) * (N // 4))
                    nc.sync.dma_start(
                        out=xf_sb[:, k, js],
                        in_=xf_d[k * P : (k + 1) * P, js],
                    )
            nc.sync.dma_start(out=consts_sb[:], in_=consts_d[:])
            for k in range(2):
                cs = slice(k * P, (k + 1) * P)
                nc.sync.dma_start(out=wqT1_sb[:, k, :], in_=wqT1_d[cs, :])
                nc.sync.dma_start(out=wkT1_sb[:, k, :], in_=wkT1_d[cs, :])
                nc.sync.dma_start(out=wvT1_sb[:, k, :], in_=wvT1_d[cs, :])
                nc.sync.dma_start(out=xc_sb[:, k, :], in_=xc_d[cs, :])
                nc.sync.dma_start(out=wqT2_sb[:, k, :], in_=wqT2_d[cs, :])
                nc.sync.dma_start(out=wkT2_sb[:, k, :], in_=wkT2_d[cs, :])
                nc.sync.dma_start(out=wvT2_sb[:, k, :], in_=wvT2_d[cs, :])
            nc.sync.dma_start(
                out=mask_sb[:], in_=mrow_d[0, :].partition_broadcast(P)
            )
            nc.sync.dma_start(
                out=maskc_sb[:], in_=mcrow_d[0, :].partition_broadcast(P)
            )
            nc.vector.memset(onesc_sb[:], 1.0)
            nc.vector.memset(onesr_sb[:].bitcast(f32), 1.0)

            def conv_qk(wT_sb, bias_col, src_of, width, out_sb):
                """out (D x width) = wT.T @ src + bias.  src_of(k, js) gives
                the (128 x 512) input-channel tile."""
                for jc in range(width // IC):
                    js = slice(jc * IC, (jc + 1) * IC)
                    ps = psA.tile([D, IC], f32, tag="a", name="qk_ps")
                    nc.tensor.matmul(
                        ps[:], wT_sb[:, 0, :], src_of(0, js),
                        start=True, stop=False,
                    )
                    nc.tensor.matmul(
                        ps[:], wT_sb[:, 1, :], src_of(1, js),
                        start=False, stop=True,
                    )
                    nc.vector.tensor_scalar_add(
                        out_sb[:, js], ps[:],
                        consts_sb[0:D, bias_col : bias_col + 1],
                    )

            def conv_vT(wvT_sb, src_of, v_sb, t):
                """v_sb[:, t, :] = (src^T @ wvT) for key tile t (j on
                partitions, channels free)."""
                ts_ = slice(t * P, (t + 1) * P)
                ps = psA.tile([P, C], f32, tag="a", name="v_ps")
                nc.tensor.matmul(
                    ps[:], src_of(0, ts_), wvT_sb[:, 0, :],
                    start=True, stop=False,
                )
                nc.tensor.matmul(
                    ps[:], src_of(1, ts_), wvT_sb[:, 1, :],
                    start=False, stop=True,
                )
                nc.vector.tensor_copy(v_sb[:, t, :], ps[:])

            def attention(q_sb, k_sb, v_sb, epilogue):
                """Row-chunk attention.  Per i-chunk: S^T = K-tile^T Q
                (j on partitions), E = exp(S^T), then O(c,i) accumulates
                with V^T slices stationary and E moving; the denominator
                row comes from an M=1 ones matmul.  The S/exp stage is
                emitted two key-tiles ahead of AV/den so the in-order PE
                queue never stalls on the ACT exp.  epilogue(ich, accs,
                rrep) gets natural-layout unnormalized O accumulators and
                the partition-replicated reciprocal denominator (SBUF)."""
                LOOKAHEAD = 2
                for ich in range(R // IC):
                    is_ = slice(ich * IC, (ich + 1) * IC)
                    accs = [
                        psO.tile([P, IC], f32, tag="o", name="acc")
                        for _ in range(2)
                    ]
                    den = psA.tile([1, IC], f32, tag="a", name="den")
                    es = {}

                    def s_exp(t):
                        sps = psS.tile([P, IC], f32, tag="s", name="s_ps")
                        nc.tensor.matmul(
                            sps[:],
                            k_sb[:, t * P : (t + 1) * P],
                            q_sb[:, is_],
                            start=True, stop=True,
                        )
                        e_sb = epool.tile([P, IC], bf16, tag="e", name="e_sb")
                        nc.scalar.activation(e_sb[:], sps[:], AF.Exp)
                        es[t] = e_sb

                    for t in range(LOOKAHEAD):
                        s_exp(t)
                    for t in range(NT):
                        if t + LOOKAHEAD < NT:
                            s_exp(t + LOOKAHEAD)
                        e_sb = es.pop(t)
                        for ct in range(2):
                            nc.tensor.matmul(
                                accs[ct][:],
                                v_sb[:, t, ct * P : (ct + 1) * P],
                                e_sb[:],
                                start=(t == 0), stop=(t == NT - 1),
                            )
                        nc.tensor.matmul(
                            den[:], onesc_sb[:], e_sb[:],
                            start=(t == 0), stop=(t == NT - 1),
                        )
                    # reciprocal of the denominator row, replicated to all
                    # partitions via a K=1 ones matmul (f32r: single-pass)
                    rrow = rcpool.tile([1, IC], f32, tag="rc", name="rrow")
                    nc.vector.reciprocal(rrow[:], den[:])
                    rrow_r = rcpool.tile([1, IC], f32r, tag="rcr", name="rrow_r")
                    nc.vector.tensor_copy(rrow_r[:], rrow[:])
                    rrep_ps = psA.tile([P, IC], f32, tag="a", name="rrep_ps")
                    nc.tensor.matmul(
                        rrep_ps[:], onesr_sb[:], rrow_r[:],
                        start=True, stop=True,
                    )
                    rrep = onpool.tile([P, IC], f32, tag="rr", name="rrep")
                    nc.scalar.copy(rrep[:], rrep_ps[:])
                    epilogue(ich, accs, rrep)

            # ================= Layer 1: self-attention =================
            q1_sb = big.tile([D, R], bf16, tag="q", name="q1_sb")
            k1_sb = big.tile([D, N], bf16, tag="k", name="k1_sb")
            v1_sb = big.tile([P, NT, C], bf16, tag="v", name="v1_sb")

            conv_qk(wqT1_sb, 6, lambda k, js: xc_sb[:, k, js], R, q1_sb)
            conv_qk(wkT1_sb, 7, lambda k, js: xf_sb[:, k, js], N, k1_sb)
            for t in range(NT):
                conv_vT(wvT1_sb, lambda k, ts_: xf_sb[:, k, ts_], v1_sb, t)

            def epilogue1(ich, accs, rrep):
                io = slice(ich * IC, (ich + 1) * IC)
                for ct in range(2):
                    # x' = sa_gamma * (O/den) + sa_gamma*bv + x, fused as
                    # ((O * sa_gamma) * rrep), then ((t + sgb) + x)
                    nc.vector.scalar_tensor_tensor(
                        xp_sb[:, ct, io], accs[ct][:],
                        consts_sb[:, 0:1], rrep[:],
                        op0=OP.mult, op1=OP.mult,
                    )
                    nc.vector.scalar_tensor_tensor(
                        xp_sb[:, ct, io], xp_sb[:, ct, io],
                        consts_sb[:, 2 + ct : 3 + ct],
                        xc_sb[:, ct, io].bitcast(f32),
                        op0=OP.add, op1=OP.add,
                    )
                    nc.vector.tensor_copy(
                        xp16_sb[:, ct, io], xp_sb[:, ct, io]
                    )

            attention(q1_sb, k1_sb, v1_sb, epilogue1)

            # ====== AllGather x' within each batch group (2 phases) ======
            # Phase h gathers x' columns [h*512, (h+1)*512) of every rank;
            # phase 0 overlaps the second layer-1 attention i-chunk.
            xpf_sb = big.tile([P, 2, N], bf16, tag="xbig", name="xpf_sb")
            for h in range(2):
                hs = slice(h * IC, (h + 1) * IC)
                ag_in = dram.tile(
                    [C, IC], bf16, tag=f"ag_in{h}", name=f"ag_in{h}"
                )
                ag_out = dram.tile(
                    [RSH, C, IC], bf16, tag=f"ag_out{h}", name=f"ag_out{h}"
                )
                for ct in range(2):
                    nc.sync.dma_start(
                        out=ag_in[ct * P : (ct + 1) * P, :],
                        in_=xp16_sb[:, ct, hs],
                    )
                nc.gpsimd.collective_compute(
                    "AllGather",
                    OP.bypass,
                    replica_groups=groups,
                    ins=[ag_in[:].opt()],
                    outs=[ag_out[:].opt()],
                )
                for ct in range(2):
                    for r in range(RSH):
                        nc.sync.dma_start(
                            out=xpf_sb[
                                :, ct, r * R + h * IC : r * R + (h + 1) * IC
                            ],
                            in_=ag_out[r, ct * P : (ct + 1) * P, :],
                        )

            # ============== Layer 2: masked cross-attention ==============
            # feature_f chunk + its per-channel stats (cols 0-3 of stats_sb)
            ffsq = misc.tile([P, R], f32, tag="ffsq", name="ffsq")
            for ct in range(2):
                nc.vector.tensor_mul(
                    ff_sb[:, ct, :], maskc_sb[:], xp_sb[:, ct, :]
                )
                nc.vector.tensor_reduce(
                    stats_sb[:, ct : ct + 1], ff_sb[:, ct, :].bitcast(f32),
                    axis=AX.X, op=OP.add,
                )
                nc.vector.tensor_mul(
                    ffsq[:],
                    ff_sb[:, ct, :].bitcast(f32),
                    ff_sb[:, ct, :].bitcast(f32),
                )
                nc.vector.tensor_reduce(
                    stats_sb[:, 2 + ct : 3 + ct], ffsq[:],
                    axis=AX.X, op=OP.add,
                )

            q2_sb = big.tile([D, R], bf16, tag="q", name="q2_sb")
            conv_qk(wqT2_sb, 8, lambda k, js: ff_sb[:, k, js], R, q2_sb)

            # feature_b tiles on the fly -> K2 and V2^T convs.  Chunk order
            # interleaves gather phases: even chunks only need AG phase 0.
            k2_sb = big.tile([D, N], bf16, tag="k", name="k2_sb")
            v2_sb = big.tile([P, NT, C], bf16, tag="v", name="v2_sb")
            for jc in (0, 2, 4, 6, 1, 3, 5, 7):
                js = slice(jc * IC, (jc + 1) * IC)
                fb0 = fbpool.tile([P, IC], bf16, tag="fb", name="fb0")
                fb1 = fbpool.tile([P, IC], bf16, tag="fb", name="fb1")
                # fb = (1-mask)*x' = x' - mask*x'
                nc.vector.tensor_mul(
                    fb0[:], mask_sb[:, js], xpf_sb[:, 0, js]
                )
                nc.vector.tensor_sub(
                    fb0[:], xpf_sb[:, 0, js], fb0[:]
                )
                nc.vector.tensor_mul(
                    fb1[:], mask_sb[:, js], xpf_sb[:, 1, js]
                )
                nc.vector.tensor_sub(
                    fb1[:], xpf_sb[:, 1, js], fb1[:]
                )
                ps = psA.tile([D, IC], f32, tag="a", name="k2_ps")
                nc.tensor.matmul(
                    ps[:], wkT2_sb[:, 0, :], fb0[:], start=True, stop=False
                )
                nc.tensor.matmul(
                    ps[:], wkT2_sb[:, 1, :], fb1[:], start=False, stop=True
                )
                nc.vector.tensor_scalar_add(
                    k2_sb[:, js], ps[:], consts_sb[0:D, 9:10]
                )
                for tsub in range(IC // P):
                    t = jc * (IC // P) + tsub
                    ts_ = slice(tsub * P, (tsub + 1) * P)
                    psv = psA.tile([P, C], f32, tag="a", name="v2_ps")
                    nc.tensor.matmul(
                        psv[:], fb0[:, ts_], wvT2_sb[:, 0, :],
                        start=True, stop=False,
                    )
                    nc.tensor.matmul(
                        psv[:], fb1[:, ts_], wvT2_sb[:, 1, :],
                        start=False, stop=True,
                    )
                    nc.vector.tensor_copy(v2_sb[:, t, :], psv[:])

            def epilogue2(ich, accs, rrep):
                # normalized sw_bg chunk in natural layout; accumulate
                # per-channel sum/sumsq into stats_sb cols 4-7 via VE
                for ct in range(2):
                    onb = onpool.tile([P, IC], f32, tag="on", name="on2")
                    s1 = rcpool.tile([P, 1], f32, tag="s1", name="s1")
                    nc.vector.scalar_tensor_tensor(
                        onb[:], accs[ct][:], 1.0, rrep[:],
                        op0=OP.mult, op1=OP.mult, accum_out=s1[:],
                    )
                    sqb = sqpool.tile([P, IC], f32, tag="sq", name="sq2")
                    s2 = rcpool.tile([P, 1], f32, tag="s2", name="s2")
                    nc.vector.scalar_tensor_tensor(
                        sqb[:], onb[:], 1.0, onb[:],
                        op0=OP.mult, op1=OP.mult, accum_out=s2[:],
                    )
                    if ich == 0:
                        nc.vector.tensor_copy(
                            stats_sb[:, 4 + ct : 5 + ct], s1[:]
                        )
                        nc.vector.tensor_copy(
                            stats_sb[:, 6 + ct : 7 + ct], s2[:]
                        )
                    else:
                        nc.vector.tensor_add(
                            stats_sb[:, 4 + ct : 5 + ct],
                            stats_sb[:, 4 + ct : 5 + ct], s1[:],
                        )
                        nc.vector.tensor_add(
                            stats_sb[:, 6 + ct : 7 + ct],
                            stats_sb[:, 6 + ct : 7 + ct], s2[:],
                        )

            attention(q2_sb, k2_sb, v2_sb, epilogue2)

            # ================== stats AllReduce + FMM ==================
            ar_in = dram.tile([P, 8], f32, tag="ar_in", name="ar_in")
            ar_out = dram.tile([P, 8], f32, tag="ar_out", name="ar_out")
            nc.sync.dma_start(out=ar_in[:], in_=stats_sb[:])
            nc.gpsimd.collective_compute(
                "AllReduce",
                OP.add,
                replica_groups=groups,
                ins=[ar_in[:].opt()],
                outs=[ar_out[:].opt()],
            )
            rst = misc.tile([P, 8], f32, tag="rst", name="rst")
            nc.sync.dma_start(out=rst[:], in_=ar_out[:])

            # var = (S2 - S1^2/N)/(N-1) + EPS (both channel-halves at once)
            varf = misc.tile([P, 2], f32, tag="varf", name="varf")
            varg = misc.tile([P, 2], f32, tag="varg", name="varg")
            ratio = misc.tile([P, 2], f32, tag="ratio", name="ratio")
            for var, s1s, s2s in ((varf, 0, 2), (varg, 4, 6)):
                nc.vector.tensor_mul(
                    var[:], rst[:, s1s : s1s + 2], rst[:, s1s : s1s + 2]
                )
                nc.vector.tensor_scalar(
                    var[:], var[:], -1.0 / N, None, op0=OP.mult
                )
                nc.vector.tensor_add(var[:], var[:], rst[:, s2s : s2s + 2])
                nc.vector.tensor_scalar(
                    var[:], var[:], 1.0 / (N - 1), EPS, op0=OP.mult, op1=OP.add
                )
            nc.vector.reciprocal(varf[:], varf[:])
            nc.vector.tensor_mul(varg[:], varg[:], varf[:])
            nc.scalar.activation(ratio[:], varg[:], AF.Sqrt)
            # fold in gamma
            nc.vector.tensor_scalar_mul(ratio[:], ratio[:], consts_sb[:, 1:2])

            # out = x' + (gamma * std_bg/std_f) * ff
            for ct in range(2):
                fin = finpool.tile([P, R], f32, tag="fin", name="fin")
                nc.vector.scalar_tensor_tensor(
                    fin[:], ff_sb[:, ct, :].bitcast(f32),
                    ratio[:, ct : ct + 1], xp_sb[:, ct, :],
                    op0=OP.mult, op1=OP.add,
                )
                nc.sync.dma_start(
                    out=out_d[ct * P : (ct + 1) * P, :], in_=fin[:]
                )

    nc.compile()
    return nc


def _prep_inputs(x, mask, sa_wq, sa_bq, sa_wk, sa_bk, sa_wv, sa_bv, sa_gamma,
                 wq, bq, wk, bk, wv, bv, gamma):
    """Build the per-core input maps (host-side sharding + weight layout)."""
    x = np.ascontiguousarray(x, dtype=F32)
    mask = np.ascontiguousarray(mask, dtype=F32)

    import ml_dtypes

    BF16 = ml_dtypes.bfloat16
    wqT1 = np.ascontiguousarray(sa_wq.T, dtype=F32)
    wkT1 = np.ascontiguousarray(sa_wk.T.astype(BF16))
    wvT1 = np.ascontiguousarray(sa_wv.T.astype(BF16))
    wqT2 = np.ascontiguousarray(wq.T, dtype=F32)
    wkT2 = np.ascontiguousarray(wk.T.astype(BF16))
    wvT2 = np.ascontiguousarray(wv.T.astype(BF16))

    consts = np.zeros((P, 10), dtype=F32)
    consts[:, 0] = sa_gamma[0]
    consts[:, 1] = gamma[0]
    sgb = (sa_gamma[0] * sa_bv).astype(F32)
    consts[:, 2] = sgb[0:P]
    consts[:, 3] = sgb[P:C]
    consts[0:D, 6] = sa_bq
    consts[0:D, 7] = sa_bk
    consts[0:D, 8] = bq
    consts[0:D, 9] = bk

    in_maps = []
    for g in range(NCORES):
        b, r = g // RSH, g % RSH
        xb = np.ascontiguousarray(x[b].reshape(C, N))
        mb = np.ascontiguousarray(mask[b].reshape(1, N))
        in_maps.append({
            "xf": np.ascontiguousarray(xb.astype(BF16)),
            "xc": np.ascontiguousarray(xb[:, r * R : (r + 1) * R]),
            "mrow": mb,
            "mcrow": np.ascontiguousarray(mb[:, r * R : (r + 1) * R]),
            "wqT1": wqT1, "wkT1": wkT1, "wvT1": wvT1,
            "wqT2": wqT2, "wkT2": wkT2, "wvT2": wvT2,
            "consts": consts,
        })
    return in_maps


def kernel(**inputs):
    from concourse import bass_utils

    if "nc" not in _CACHE:
        _CACHE["nc"] = _build_bass()
    nc = _CACHE["nc"]

    in_maps = _prep_inputs(**inputs)
    res = bass_utils.run_bass_kernel_spmd(
        nc, in_maps, core_ids=list(range(NCORES))
    )
    _CACHE["last_results"] = res

    out = np.empty((B, C, N), dtype=F32)
    for g in range(NCORES):
        b, r = g // RSH, g % RSH
        out[b, :, r * R : (r + 1) * R] = res.results[g]["outc"]
    return out.reshape(B, C, HH, WW)



# revision 28
# speedup vs baseline: 1.1923x; 1.1923x over previous
"""Trainium2 Bass/Tile kernel for nn_FB_FMM (sparse_attention), v2.

Computation (per batch element b, N = H*W = 4096 tokens, C=256, D=32):
  1. Self-attention:  sa_out = attn(conv(x,sa_wq), conv(x,sa_wk), conv(x,sa_wv))
     x' = sa_gamma * sa_out + x
  2. Masked cross-attention (FB_FMM):
     ff = mask * x'; fb = (1-mask) * x'
     sw_bg = attn(conv(ff,wq), conv(fb,wk), conv(fb,wv))
     out = x' + gamma * ff * (std(sw_bg)/std(ff))    [per-channel std, ddof=1]

Sharding: 8 cores = 2 batch groups x 4-way query-row sharding (1024 rows/core).

v2 design notes:
  - K-conv biases dropped: a key bias shifts all logits of a query row by
    q.bk, which cancels in the row softmax.
  - S^T computed transposed (keys on partitions); exp on ACT with a
    compile-time logit shift, writing fp8e5 pairs; AV + denominator matmuls
    run in fp8 DoubleRow (2 key tiles per pass, 0.5 cycles/row); V tiles in
    fp8e4.  sw_bg only feeds per-channel variance so layer-2 fp8 noise
    averages out over N=4096.
  - Each core convolves its own row chunk into K2/V2^T (fp8) and AllGathers
    those (144KB/phase, 2 phases) instead of gathering x' and redoing the
    convs; attention-2 accumulates phase-A key tiles while phase-B's
    AllGather is in flight.
  - Softmax denominators share one PSUM bank at partition rows 0/32/64/96;
    reciprocal via the fast custom-DVE approx; the reciprocal row is
    replicated across partitions with a K=1 ones matmul and read directly
    from PSUM by the epilogue.
  - Per-channel [sum, sumsq] of ff and sw_bg exchanged with one small
    AllGather at the end (+3 local adds); a dummy warmup collective at
    kernel start absorbs the one-time CC barrier / core skew.
  - Inputs DMAed on 4 hardware queues, highest-priority tensors first.
"""

import numpy as np

P = 128
B, C, HH, WW = 2, 256, 64, 64
N = HH * WW            # 4096 tokens
D = 32                 # q/k channels
NCORES = 8
RSH = 4                # row shards per batch group
R = N // RSH           # 1024 query rows per core
NT = N // P            # 32 key tiles per layer
IC = 512               # query i-chunk (one PSUM bank of fp32)
EPS = 1e-5
F32 = np.float32

FP8_L1 = True          # AV1/den1 fp8 DoubleRow (E1 fp8e5, V1 fp8e4)
FP8_L2 = True          # AV2/den2 fp8 DoubleRow (E2 fp8e5, V2/K2 fp8e4)
SHIFT1 = 13.5          # global logit shift subtracted inside exp (layer 1)
SHIFT2 = 14.0          # per-foreground-query logit shift (layer 2), folded
                       # into the S2 matmul via an extra contraction channel

_CACHE = {}


def _build_bass():
    import concourse.bass as bass
    from concourse import bacc, mybir, tile

    f32 = mybir.dt.float32
    f32r = mybir.dt.float32r
    bf16 = mybir.dt.bfloat16
    fp8e4 = mybir.dt.float8e4
    fp8e5 = mybir.dt.float8e5
    u8 = mybir.dt.uint8
    OP = mybir.AluOpType
    AF = mybir.ActivationFunctionType
    DR = mybir.MatmulPerfMode.DoubleRow

    nc = bacc.Bacc(
        "TRN2", target_bir_lowering=False, debug=False, num_devices=NCORES
    )

    e1_dt = fp8e5 if FP8_L1 else bf16
    v1_dt = fp8e4 if FP8_L1 else bf16
    e2_dt = fp8e5 if FP8_L2 else bf16
    v2_dt = fp8e4 if FP8_L2 else bf16
    k2_dt = v2_dt
    q2_dt = v2_dt  # S2 runs fp8 x fp8; layer 2 only feeds variance stats

    # ---------------- I/O ----------------
    xf_d = nc.dram_tensor("xf", [C, N], bf16, kind="ExternalInput")
    xc_d = nc.dram_tensor("xc", [C, R], f32r, kind="ExternalInput")
    mcrow_d = nc.dram_tensor("mcrow", [1, R], f32, kind="ExternalInput")
    wqT1_d = nc.dram_tensor("wqT1", [C, D], f32r, kind="ExternalInput")
    wkT1_d = nc.dram_tensor("wkT1", [C, D], bf16, kind="ExternalInput")
    wvT1_d = nc.dram_tensor("wvT1", [C, C], bf16, kind="ExternalInput")
    wqT2_d = nc.dram_tensor("wqT2", [C, D], f32r, kind="ExternalInput")
    wkT2_d = nc.dram_tensor("wkT2", [C, D], bf16, kind="ExternalInput")
    wvT2_d = nc.dram_tensor("wvT2", [C, C], bf16, kind="ExternalInput")
    # consts: col 0 sa_gamma, 1 gamma, 2/3 sa_gamma*sa_bv halves,
    # 6 sa_bq, 8 bq (cols 6/8 live on partitions 0..31)
    consts_d = nc.dram_tensor("consts", [P, 10], f32, kind="ExternalInput")
    out_d = nc.dram_tensor("outc", [C, R], f32, kind="ExternalOutput")

    groups = [[0, 1, 2, 3], [4, 5, 6, 7]]

    # AG payload per phase, in k2_dt elements: K2 [D, IC] + V2T [IC, C],
    # both row-blocked to C columns -> [(D*IC + IC*C)/C, C]
    K2R = D * IC // C      # 64 rows of C
    V2R = IC               # 512 rows of C
    AGR = K2R + V2R

    with tile.TileContext(nc) as tc:
        from contextlib import ExitStack

        ctx = ExitStack()
        with ctx:
            big = ctx.enter_context(tc.tile_pool(name="big", bufs=1))
            epool = ctx.enter_context(tc.tile_pool(name="epool", bufs=4))
            sqpool = ctx.enter_context(tc.tile_pool(name="sqpool", bufs=2))
            fbpool = ctx.enter_context(tc.tile_pool(name="fbpool", bufs=2))
            rcpool = ctx.enter_context(tc.tile_pool(name="rcpool", bufs=2))
            finpool = ctx.enter_context(tc.tile_pool(name="finpool", bufs=2))
            misc = ctx.enter_context(tc.tile_pool(name="misc", bufs=1))
            psA = ctx.enter_context(
                tc.tile_pool(name="psA", bufs=2, space="PSUM")
            )
            psS = ctx.enter_context(
                tc.tile_pool(name="psS", bufs=2, space="PSUM")
            )
            psO = ctx.enter_context(
                tc.tile_pool(name="psO", bufs=2, space="PSUM")
            )
            psD = ctx.enter_context(
                tc.tile_pool(name="psD", bufs=2, space="PSUM")
            )
            dram = ctx.enter_context(
                tc.tile_pool(name="dram", bufs=1, space="DRAM")
            )

            # ------------- persistent SBUF tiles -------------
            xf_sb = big.tile([P, 2, N], bf16, tag="xbig", name="xf_sb")
            xc_sb = big.tile([P, 2, R], f32r, tag="xc", name="xc_sb")
            maskc_sb = big.tile([P, R], f32, tag="maskc", name="maskc_sb")
            xp_sb = big.tile([P, 2, R], f32, tag="xp", name="xp_sb")
            ff_sb = big.tile([P, 2, R], f32r, tag="ff", name="ff_sb")
            wqT1_sb = big.tile([P, 2, D], f32r, tag="wqT1", name="wqT1_sb")
            wkT1_sb = big.tile([P, 2, D], bf16, tag="wkT1", name="wkT1_sb")
            wvT1_sb = big.tile([P, 2, C], bf16, tag="wvT1", name="wvT1_sb")
            wqT2_sb = big.tile([P, 2, D], f32r, tag="wqT2", name="wqT2_sb")
            wkT2_sb = big.tile([P, 2, D], bf16, tag="wkT2", name="wkT2_sb")
            wvT2_sb = big.tile([P, 2, C], bf16, tag="wvT2", name="wvT2_sb")
            consts_sb = big.tile([P, 10], f32, tag="consts", name="consts_sb")
            # dual-fp8 ldweights needs the k-pair stride 16B-aligned, so the
            # ones column pair is padded to stride 16
            ones8_sb = big.tile([P, 2, 16], fp8e4, tag="ones8",
                                name="ones8_sb")
            onesc_sb = big.tile([P, 1], bf16, tag="onesc", name="onesc_sb")
            onesr_sb = big.tile([1, P], f32r, tag="onesr", name="onesr_sb")
            stats_sb = misc.tile([P, 8], f32, tag="stats", name="stats_sb")
            wu_sb = misc.tile([1, 4], f32, tag="wu", name="wu_sb")
            sh1_sb = misc.tile([P, 1], f32, tag="sh1", name="sh1_sb")
            sh0_sb = misc.tile([P, 1], f32, tag="sh0", name="sh0_sb")

            q1_sb = big.tile([D, R], bf16, tag="q1", name="q1_sb")
            k1_sb = big.tile([D, N], bf16, tag="k1", name="k1_sb")
            v1_sb = big.tile([P, NT, C], v1_dt, tag="v1", name="v1_sb")
            # row D of q2 carries -SHIFT2*mask_i; row D of k2 is constant
            # 1.0, so the S2 matmul computes q2.k2 - shift_i directly (the
            # bimodal layer-2 logit range cannot fit fp8e5 under any global
            # shift: background-query rows peak near 0, foreground near 21)
            q2_sb = big.tile([D + 1, R], q2_dt, tag="q2", name="q2_sb")
            k2_sb = big.tile([D + 1, NT * P], k2_dt, tag="k2", name="k2_sb")
            v2_sb = big.tile([P, NT, C], v2_dt, tag="v2", name="v2_sb")

            # softmax denominators: DoubleRow matmuls require dst partition
            # 0, so each den gets its own rotating PSUM bank slot
            dens = {}

            def den_tile(key):
                if key not in dens:
                    dens[key] = psD.tile(
                        [1, IC], f32, tag="den", name=f"den_{key}"
                    )
                return dens[key][:]

            # --------- input DMAs: 4 queues, priority order ---------
            nc.sync.dma_start(out=consts_sb[:], in_=consts_d[:])
            for k in range(2):
                cs = slice(k * P, (k + 1) * P)
                nc.sync.dma_start(out=wqT1_sb[:, k, :], in_=wqT1_d[cs, :])
            for k in range(2):
                cs = slice(k * P, (k + 1) * P)
                nc.sync.dma_start(out=xc_sb[:, k, :], in_=xc_d[cs, :])
            for k in range(2):
                cs = slice(k * P, (k + 1) * P)
                nc.sync.dma_start(out=wkT1_sb[:, k, :], in_=wkT1_d[cs, :])
                nc.sync.dma_start(out=wvT1_sb[:, k, :], in_=wvT1_d[cs, :])
            for k in range(2):
                cs = slice(k * P, (k + 1) * P)
                nc.sync.dma_start(out=wqT2_sb[:, k, :], in_=wqT2_d[cs, :])
                nc.sync.dma_start(out=wkT2_sb[:, k, :], in_=wkT2_d[cs, :])
                nc.sync.dma_start(out=wvT2_sb[:, k, :], in_=wvT2_d[cs, :])
            qeng = [nc.scalar, nc.gpsimd]
            for jc in range(8):
                js = slice(jc * IC, (jc + 1) * IC)
                eng = qeng[jc % 2]
                for k in range(2):
                    eng.dma_start(
                        out=xf_sb[:, k, js], in_=xf_d[k * P : (k + 1) * P, js]
                    )
            nc.gpsimd.dma_start(
                out=maskc_sb[:], in_=mcrow_d[0, :].partition_broadcast(P)
            )

            nc.vector.memset(ones8_sb[:].bitcast(u8), 0x38)  # fp8e4 1.0
            if FP8_L2:
                nc.vector.memset(k2_sb[D : D + 1, :].bitcast(u8), 0x38)
            nc.vector.memset(onesc_sb[:], 1.0)
            nc.vector.memset(onesr_sb[:].bitcast(f32), 1.0)
            nc.vector.memset(wu_sb[:], 0.0)
            nc.vector.memset(sh1_sb[:], -SHIFT1)
            nc.vector.memset(sh0_sb[:], 0.0)

            # warmup collective: absorbs one-time CC barrier + core skew
            wu_in = dram.tile([1, 4], f32, tag="wu_in", name="wu_in")
            wu_out = dram.tile([RSH, 4], f32, tag="wu_out", name="wu_out")
            nc.sync.dma_start(out=wu_in[:], in_=wu_sb[:])
            nc.gpsimd.collective_compute(
                "AllGather", OP.bypass, replica_groups=groups,
                ins=[wu_in[:].opt()], outs=[wu_out[:].opt()],
            )

            # ---------------- layer-1 convs (PSUM: psO) ----------------
            def conv_qk(pool, wT_sb, bias_col, src_of, width, out_sb, col0=0):
                for jc in range(width // IC):
                    js = slice(jc * IC, (jc + 1) * IC)
                    jso = slice(col0 + jc * IC, col0 + (jc + 1) * IC)
                    ps = pool.tile([D, IC], f32, tag=pool.name[-1], name="qk")
                    nc.tensor.matmul(
                        ps[:], wT_sb[:, 0, :], src_of(0, js),
                        start=True, stop=False,
                    )
                    nc.tensor.matmul(
                        ps[:], wT_sb[:, 1, :], src_of(1, js),
                        start=False, stop=True,
                    )
                    if bias_col is None:
                        nc.vector.tensor_copy(out_sb[:, jso], ps[:])
                    else:
                        nc.vector.tensor_scalar_add(
                            out_sb[:, jso], ps[:],
                            consts_sb[0:D, bias_col : bias_col + 1],
                        )

            conv_qk(psO, wqT1_sb, 6, lambda k, js: xc_sb[:, k, js], R, q1_sb)
            conv_qk(psO, wkT1_sb, None, lambda k, js: xf_sb[:, k, js], N,
                    k1_sb)
            for t in range(NT):
                ts_ = slice(t * P, (t + 1) * P)
                ps = psO.tile([P, C], f32, tag="O", name="v1_ps")
                nc.tensor.matmul(
                    ps[:], xf_sb[:, 0, ts_], wvT1_sb[:, 0, :],
                    start=True, stop=False,
                )
                nc.tensor.matmul(
                    ps[:], xf_sb[:, 1, ts_], wvT1_sb[:, 1, :],
                    start=False, stop=True,
                )
                nc.vector.tensor_copy(v1_sb[:, t, :], ps[:])

            # ---------------- generic attention pipeline ----------------
            def attention_run(items, after_cb, fp8, e_dt, shift, k_sb, v_sb,
                              q_of, acc_of, den_of, nm):
                """items: (ich, t0, start, stop).  S/exp one pair ahead of
                AV/den; after_cb(j) called after item j's AV/den."""

                def s_exp(it):
                    ich, t0, _, _ = it
                    ep = epool.tile([P, 2, IC], e_dt, tag="e", name=f"e{nm}")
                    for h in range(2):
                        t = t0 + h
                        sps = psS.tile([P, IC], f32, tag="S", name=f"s{nm}")
                        nc.tensor.matmul(
                            sps[:], k_sb[:, t * P : (t + 1) * P], q_of(ich),
                            start=True, stop=True,
                        )
                        nc.scalar.activation(
                            ep[:, h, :], sps[:], AF.Exp, bias=shift[:]
                        )
                    return ep

                eps = {0: s_exp(items[0])}
                for j, it in enumerate(items):
                    if j + 1 < len(items):
                        eps[j + 1] = s_exp(items[j + 1])
                    ich, t0, st, sp = it
                    ep = eps.pop(j)
                    if fp8:
                        for ct in range(2):
                            nc.tensor.matmul(
                                acc_of(ich, ct),
                                v_sb[:, t0 : t0 + 2, ct * P : (ct + 1) * P],
                                ep[:],
                                start=st, stop=sp, perf_mode=DR,
                            )
                        nc.tensor.matmul(
                            den_of(ich), ones8_sb[:, :, 0:1], ep[:],
                            start=st, stop=sp, perf_mode=DR,
                            skip_group_check=True,
                        )
                    else:
                        for h in range(2):
                            st_h, sp_h = st and h == 0, sp and h == 1
                            t = t0 + h
                            for ct in range(2):
                                nc.tensor.matmul(
                                    acc_of(ich, ct),
                                    v_sb[:, t, ct * P : (ct + 1) * P],
                                    ep[:, h, :],
                                    start=st_h, stop=sp_h,
                                )
                            nc.tensor.matmul(
                                den_of(ich), onesc_sb[:], ep[:, h, :],
                                start=st_h, stop=sp_h,
                                skip_group_check=True,
                            )
                    after_cb(j)

            def rrep_mm(den_slice):
                rrow = rcpool.tile([1, IC], f32, tag="rc", name="rrow")
                nc.vector.reciprocal_approx_fast(rrow[:], den_slice)
                rrow_r = rcpool.tile([1, IC], f32r, tag="rcr", name="rrow_r")
                nc.vector.tensor_copy(rrow_r[:], rrow[:])
                rrep_ps = psS.tile([P, IC], f32, tag="S", name="rrep_ps")
                nc.tensor.matmul(
                    rrep_ps[:], onesr_sb[:], rrow_r[:],
                    start=True, stop=True,
                )
                # epilogue STTs read acc from PSUM; DVE allows only one
                # PSUM operand, so stage rrep in SBUF
                rrep = rcpool.tile([P, IC], f32, tag="rrep", name="rrep")
                nc.vector.tensor_copy(rrep[:], rrep_ps[:])
                return rrep

            # ---------------- layer 1 attention ----------------
            NPAIR = NT // 2
            l1_items = []
            for ich in range(2):
                for p in range(NPAIR):
                    l1_items.append((ich, 2 * p, p == 0, p == NPAIR - 1))

            accs1 = {}

            def acc1_of(ich, ct):
                key = (ich, ct)
                if key not in accs1:
                    accs1[key] = psO.tile(
                        [P, IC], f32, tag="O", name=f"acc1_{ich}_{ct}"
                    )
                return accs1[key][:]

            def den1_of(ich):
                return den_tile(("l1", ich))

            ag_ins, ag_outs = [], []
            for h in range(2):
                ag_ins.append(dram.tile(
                    [AGR, C], k2_dt, tag=f"agi{h}", name=f"agi{h}"
                ))
                ag_outs.append(dram.tile(
                    [RSH, AGR, C], k2_dt, tag=f"ago{h}", name=f"ago{h}"
                ))

            rreps1, fbs_store = {}, {}

            def epilogue1_dve(ich):
                io = slice(ich * IC, (ich + 1) * IC)
                rrep = rreps1[ich]
                for ct in range(2):
                    nc.vector.scalar_tensor_tensor(
                        xp_sb[:, ct, io], acc1_of(ich, ct),
                        consts_sb[:, 0:1], rrep[:],
                        op0=OP.mult, op1=OP.mult,
                    )
                    nc.vector.scalar_tensor_tensor(
                        xp_sb[:, ct, io], xp_sb[:, ct, io],
                        consts_sb[:, 2 + ct : 3 + ct],
                        xc_sb[:, ct, io].bitcast(f32),
                        op0=OP.add, op1=OP.add,
                    )
                fbs = []
                for ct in range(2):
                    s1 = rcpool.tile([P, 1], f32, tag="s1", name="s1")
                    nc.vector.scalar_tensor_tensor(
                        ff_sb[:, ct, io], maskc_sb[:, io], 1.0,
                        xp_sb[:, ct, io], op0=OP.mult, op1=OP.mult,
                        accum_out=s1[:],
                    )
                    sq = sqpool.tile([P, IC], f32, tag="sq", name="sq")
                    s2 = rcpool.tile([P, 1], f32, tag="s2", name="s2")
                    nc.vector.scalar_tensor_tensor(
                        sq[:], ff_sb[:, ct, io].bitcast(f32), 1.0,
                        ff_sb[:, ct, io].bitcast(f32),
                        op0=OP.mult, op1=OP.mult, accum_out=s2[:],
                    )
                    fb = fbpool.tile([P, IC], bf16, tag="fb", name="fb")
                    nc.vector.scalar_tensor_tensor(
                        fb[:], ff_sb[:, ct, io].bitcast(f32), -1.0,
                        xp_sb[:, ct, io], op0=OP.mult, op1=OP.add,
                    )
                    fbs.append(fb)
                    if ich == 0:
                        nc.vector.tensor_copy(stats_sb[:, ct : ct + 1], s1[:])
                        nc.vector.tensor_copy(
                            stats_sb[:, 2 + ct : 3 + ct], s2[:]
                        )
                    else:
                        nc.vector.tensor_add(
                            stats_sb[:, ct : ct + 1],
                            stats_sb[:, ct : ct + 1], s1[:],
                        )
                        nc.vector.tensor_add(
                            stats_sb[:, 2 + ct : 3 + ct],
                            stats_sb[:, 2 + ct : 3 + ct], s2[:],
                        )
                return fbs

            def epilogue1_pe(ich):
                """Q2/K2own/V2own convs for chunk ich, AG ship + readback."""
                fbs = fbs_store[ich]
                io = slice(ich * IC, (ich + 1) * IC)
                conv_qk(
                    psA, wqT2_sb, 8,
                    lambda k, js: ff_sb[:, k, io],
                    IC, q2_sb[0:D, :], col0=ich * IC,
                )
                if FP8_L2:
                    nc.vector.tensor_scalar(
                        q2_sb[D : D + 1, ich * IC : (ich + 1) * IC],
                        maskc_sb[0:1, io], -SHIFT2, None, op0=OP.mult,
                    )
                ps2 = psA.tile([D, IC], f32, tag="A", name="k2_ps")
                nc.tensor.matmul(
                    ps2[:], wkT2_sb[:, 0, :], fbs[0][:],
                    start=True, stop=False,
                )
                nc.tensor.matmul(
                    ps2[:], wkT2_sb[:, 1, :], fbs[1][:],
                    start=False, stop=True,
                )
                k2own = fbpool.tile([D, 2, C], k2_dt, tag="k2o", name="k2o")
                nc.vector.tensor_copy(
                    k2own[:], ps2[:].rearrange("d (two c) -> d two c", c=C)
                )
                v2own = []
                for tsub in range(4):
                    ts_ = slice(tsub * P, (tsub + 1) * P)
                    psv = psA.tile([P, C], f32, tag="A", name="v2_ps")
                    nc.tensor.matmul(
                        psv[:], fbs[0][:, ts_], wvT2_sb[:, 0, :],
                        start=True, stop=False,
                    )
                    nc.tensor.matmul(
                        psv[:], fbs[1][:, ts_], wvT2_sb[:, 1, :],
                        start=False, stop=True,
                    )
                    vo = sqpool.tile([P, C], v2_dt, tag="v2o", name="v2o")
                    nc.vector.tensor_copy(vo[:], psv[:])
                    v2own.append(vo)
                agi, ago = ag_ins[ich], ag_outs[ich]
                nc.sync.dma_start(
                    out=agi[0:K2R, :]
                    .rearrange("(d two) c -> d two c", two=2),
                    in_=k2own[:],
                )
                for tsub in range(4):
                    r0 = K2R + tsub * P
                    nc.sync.dma_start(
                        out=agi[r0 : r0 + P, :], in_=v2own[tsub][:]
                    )
                nc.gpsimd.collective_compute(
                    "AllGather", OP.bypass, replica_groups=groups,
                    ins=[agi[:].opt()], outs=[ago[:].opt()],
                )
                for r in range(RSH):
                    cb = (ich * 16 + r * 4) * P
                    nc.sync.dma_start(
                        out=k2_sb[0:D, cb : cb + IC]
                        .rearrange("d (two c) -> d two c", c=C),
                        in_=ago[r, 0:K2R, :]
                        .rearrange("(d two) c -> d two c", two=2),
                    )
                    tb = ich * 16 + r * 4
                    nc.sync.dma_start(
                        out=v2_sb[:, tb : tb + 4, :],
                        in_=ago[r, K2R:AGR, :]
                        .rearrange("(t p) c -> p t c", p=P),
                    )

            def after1(j):
                ich, t0, st, sp = l1_items[j]
                if ich == 0 and sp:
                    rreps1[0] = rrep_mm(den1_of(0))
                    fbs_store[0] = epilogue1_dve(0)
                if ich == 1 and t0 == 6:
                    epilogue1_pe(0)

            attention_run(
                l1_items, after1, FP8_L1, e1_dt, sh1_sb, k1_sb, v1_sb,
                lambda ich: q1_sb[:, ich * IC : (ich + 1) * IC],
                acc1_of, den1_of, "1",
            )
            rreps1[1] = rrep_mm(den1_of(1))
            fbs_store[1] = epilogue1_dve(1)
            epilogue1_pe(1)

            # ---------------- layer 2 attention ----------------
            # key-tile order is phase-major: tile = phase*16 + r*4 + s.
            # ich-major item order so only one chunk's accumulators live at
            # a time; chunk 0's segment A overlaps the phase-1 AllGather.
            l2_items = []
            for ich in range(2):
                for seg in range(2):
                    for p in range(8):
                        l2_items.append(
                            (ich, seg * 16 + 2 * p,
                             seg == 0 and p == 0, seg == 1 and p == 7)
                        )

            accs2 = {}

            def acc2_of(ich, ct):
                key = (ich, ct)
                if key not in accs2:
                    accs2[key] = psO.tile(
                        [P, IC], f32, tag="O", name=f"acc2_{ich}_{ct}"
                    )
                return accs2[key][:]

            def den2_of(ich):
                return den_tile(("l2", ich))

            rreps2 = {}

            def epilogue2(ich):
                rrep = rreps2[ich]
                for ct in range(2):
                    onb = sqpool.tile([P, IC], f32, tag="sq", name="onb")
                    s1 = rcpool.tile([P, 1], f32, tag="s1", name="s1b")
                    nc.vector.scalar_tensor_tensor(
                        onb[:], acc2_of(ich, ct), 1.0, rrep[:],
                        op0=OP.mult, op1=OP.mult, accum_out=s1[:],
                    )
                    sqb = sqpool.tile([P, IC], f32, tag="sq", name="sqb")
                    s2 = rcpool.tile([P, 1], f32, tag="s2", name="s2b")
                    nc.vector.scalar_tensor_tensor(
                        sqb[:], onb[:], 1.0, onb[:],
                        op0=OP.mult, op1=OP.mult, accum_out=s2[:],
                    )
                    if ich == 0:
                        nc.vector.tensor_copy(
                            stats_sb[:, 4 + ct : 5 + ct], s1[:]
                        )
                        nc.vector.tensor_copy(
                            stats_sb[:, 6 + ct : 7 + ct], s2[:]
                        )
                    else:
                        nc.vector.tensor_add(
                            stats_sb[:, 4 + ct : 5 + ct],
                            stats_sb[:, 4 + ct : 5 + ct], s1[:],
                        )
                        nc.vector.tensor_add(
                            stats_sb[:, 6 + ct : 7 + ct],
                            stats_sb[:, 6 + ct : 7 + ct], s2[:],
                        )

            def after2(j):
                ich, t0, st, sp = l2_items[j]
                if ich == 0 and sp:
                    rreps2[0] = rrep_mm(den2_of(0))
                    epilogue2(0)

            attention_run(
                l2_items, after2, FP8_L2, e2_dt, sh0_sb, k2_sb, v2_sb,
                lambda ich: q2_sb[:, ich * IC : (ich + 1) * IC],
                acc2_of, den2_of, "2",
            )
            rreps2[1] = rrep_mm(den2_of(1))
            epilogue2(1)

            # ---------------- stats AllGather + FMM tail ----------------
            st_in = dram.tile([P, 8], f32, tag="st_in", name="st_in")
            st_out = dram.tile([RSH, P, 8], f32, tag="st_out", name="st_out")
            nc.sync.dma_start(out=st_in[:], in_=stats_sb[:])
            nc.gpsimd.collective_compute(
                "AllGather", OP.bypass, replica_groups=groups,
                ins=[st_in[:].opt()], outs=[st_out[:].opt()],
            )
            rst = misc.tile([P, 8], f32, tag="rst", name="rst")
            parts = misc.tile([P, 3, 8], f32, tag="rparts", name="rparts")
            nc.sync.dma_start(out=rst[:], in_=st_out[0])
            for r in range(1, RSH):
                nc.sync.dma_start(out=parts[:, r - 1, :], in_=st_out[r])
            for r in range(3):
                nc.vector.tensor_add(rst[:], rst[:], parts[:, r, :])

            # var = (S2 - S1^2/N)/(N-1) + EPS for ff (cols 0-3), bg (4-7)
            varf = misc.tile([P, 2], f32, tag="varf", name="varf")
            varg = misc.tile([P, 2], f32, tag="varg", name="varg")
            ratio = misc.tile([P, 2], f32, tag="ratio", name="ratio")
            scr = misc.tile([P, 2], f32, tag="scr", name="scr")
            for var, s1s, s2s in ((varf, 0, 2), (varg, 4, 6)):
                nc.vector.tensor_mul(
                    var[:], rst[:, s1s : s1s + 2], rst[:, s1s : s1s + 2]
                )
                nc.vector.tensor_scalar(
                    var[:], var[:], -1.0 / N, None, op0=OP.mult
                )
                nc.vector.tensor_add(var[:], var[:], rst[:, s2s : s2s + 2])
                nc.vector.tensor_scalar(
                    var[:], var[:], 1.0 / (N - 1), EPS, op0=OP.mult,
                    op1=OP.add,
                )
            nc.vector.reciprocal_approx_fast(scr[:], varf[:])
            nc.vector.tensor_mul(varg[:], varg[:], scr[:])
            nc.scalar.activation(ratio[:], varg[:], AF.Sqrt)
            nc.vector.tensor_scalar_mul(ratio[:], ratio[:], consts_sb[:, 1:2])

            # out = x' + (gamma * std_bg/std_f) * ff, 4 pipelined chunks
            oeng = [nc.sync, nc.scalar, nc.sync, nc.scalar]
            for ct in range(2):
                for hc in range(2):
                    io = slice(hc * IC, (hc + 1) * IC)
                    fin = finpool.tile([P, IC], f32, tag="fin", name="fin")
                    nc.vector.scalar_tensor_tensor(
                        fin[:], ff_sb[:, ct, io].bitcast(f32),
                        ratio[:, ct : ct + 1], xp_sb[:, ct, io],
                        op0=OP.mult, op1=OP.add,
                    )
                    oeng[ct * 2 + hc].dma_start(
                        out=out_d[ct * P : (ct + 1) * P, io], in_=fin[:]
                    )

    nc.compile()
    return nc


def _prep_inputs(x, mask, sa_wq, sa_bq, sa_wk, sa_bk, sa_wv, sa_bv, sa_gamma,
                 wq, bq, wk, bk, wv, bv, gamma):
    x = np.ascontiguousarray(x, dtype=F32)
    mask = np.ascontiguousarray(mask, dtype=F32)

    import ml_dtypes

    BF16 = ml_dtypes.bfloat16
    wqT1 = np.ascontiguousarray(sa_wq.T, dtype=F32)
    wkT1 = np.ascontiguousarray(sa_wk.T.astype(BF16))
    wvT1 = np.ascontiguousarray(sa_wv.T.astype(BF16))
    wqT2 = np.ascontiguousarray(wq.T, dtype=F32)
    wkT2 = np.ascontiguousarray(wk.T.astype(BF16))
    wvT2 = np.ascontiguousarray(wv.T.astype(BF16))

    consts = np.zeros((P, 10), dtype=F32)
    consts[:, 0] = sa_gamma[0]
    consts[:, 1] = gamma[0]
    sgb = (sa_gamma[0] * sa_bv).astype(F32)
    consts[:, 2] = sgb[0:P]
    consts[:, 3] = sgb[P:C]
    consts[0:D, 6] = sa_bq
    consts[0:D, 8] = bq

    in_maps = []
    for g in range(NCORES):
        b, r = g // RSH, g % RSH
        xb = np.ascontiguousarray(x[b].reshape(C, N))
        mb = np.ascontiguousarray(mask[b].reshape(1, N))
        in_maps.append({
            "xf": np.ascontiguousarray(xb.astype(BF16)),
            "xc": np.ascontiguousarray(xb[:, r * R : (r + 1) * R]),
            "mcrow": np.ascontiguousarray(mb[:, r * R : (r + 1) * R]),
            "wqT1": wqT1, "wkT1": wkT1, "wvT1": wvT1,
            "wqT2": wqT2, "wkT2": wkT2, "wvT2": wvT2,
            "consts": consts,
        })
    return in_maps


def kernel(**inputs):
    from concourse import bass_utils

    if "nc" not in _CACHE:
        _CACHE["nc"] = _build_bass()
    nc = _CACHE["nc"]

    in_maps = _prep_inputs(**inputs)
    res = bass_utils.run_bass_kernel_spmd(
        nc, in_maps, core_ids=list(range(NCORES))
    )
    _CACHE["last_results"] = res

    out = np.empty((B, C, N), dtype=F32)
    for g in range(NCORES):
        b, r = g // RSH, g % RSH
        out[b, :, r * R : (r + 1) * R] = res.results[g]["outc"]
    return out.reshape(B, C, HH, WW)
